# revision 3
# baseline (speedup 1.0000x reference)
"""Causal self-attention (B=2, T=2048, C=1024, H=16) on 8 trn2 NeuronCores.

Sharding: core c = (batch b = c // 4, head-group g = c % 4). Each core
computes, for its batch, QKV for heads [4g, 4g+4), causal attention, and a
partial output projection through rows [256g, 256g+256) of W_proj. The host
sums the 4 partial projections per batch (tensor-parallel unshard) and adds
b_proj.

Per-core kernel layout choices:
  - qk^T is produced transposed ([channel, t]) so S = Q K^T needs no input
    transposes (contraction over d=64 sits on the partition axis).
  - Two heads are row-packed per S matmul (K=64 each at array rows 0/64) and
    col-packed per PV matmul (M=64 each at array cols 0/64).
  - exp runs on ACT reading PSUM, with accum_out producing the softmax row
    sums; P is normalized in [q, k] layout with a per-partition scalar
    multiply, then PE-transposed for the PV matmul.
  - y is accumulated transposed ([d, q]) which is exactly the lhsT the
    projection needs.
"""

import sys

for _p in ("/opt/trn_rl_repo",):
    if _p not in sys.path:
        sys.path.insert(0, _p)

import numpy as np
import ml_dtypes

import concourse.bass as bass
import concourse.tile as tile
from concourse import bacc, mybir
from concourse.bass_utils import run_bass_kernel_spmd

BF16 = mybir.dt.bfloat16
F32 = mybir.dt.float32
NP_BF16 = ml_dtypes.bfloat16

B, T, C = 2, 2048, 1024
H, D = 16, 64
N_CORES = 8
HEADS_PER_CORE = 4  # 2 pairs
CT = C // 128  # 8 contraction tiles
TQ = T // 128  # 16 query blocks
SCALE = 1.0 / np.sqrt(D)

_compiled = None


def _build_nc():
    nc = bacc.Bacc("TRN2", target_bir_lowering=False, debug=False,
                   enable_asserts=False)

    xT_d = nc.dram_tensor("xT", [C, T], BF16, kind="ExternalInput")
    wqk_d = nc.dram_tensor("wqk", [C, 512], BF16, kind="ExternalInput")
    wv_d = nc.dram_tensor("wv", [C, 256], BF16, kind="ExternalInput")
    wp_d = nc.dram_tensor("wp", [256, C], BF16, kind="ExternalInput")
    bqk_d = nc.dram_tensor("bqk", [128, 4], F32, kind="ExternalInput")
    bv_d = nc.dram_tensor("bv", [128, 256], BF16, kind="ExternalInput")
    mask_d = nc.dram_tensor("maskneg", [128, 128], F32, kind="ExternalInput")
    ident_d = nc.dram_tensor("ident", [128, 128], BF16, kind="ExternalInput")
    out_d = nc.dram_tensor("out", [T, C], F32, kind="ExternalOutput")

    Exp = mybir.ActivationFunctionType.Exp
    X = mybir.AxisListType.X

    with tile.TileContext(nc) as tc:
        with (
            tc.tile_pool(name="const", bufs=1) as cpool,
            tc.tile_pool(name="qkT", bufs=1) as qkpool,
            tc.tile_pool(name="vbuf", bufs=1) as vpool,
            tc.tile_pool(name="ybuf", bufs=1) as ypool,
            tc.tile_pool(name="p", bufs=4) as ppool,
            tc.tile_pool(name="pt", bufs=10) as ptpool,
            tc.tile_pool(name="lsc", bufs=8) as lpool,
            tc.tile_pool(name="ostage", bufs=3) as opool,
            tc.tile_pool(name="mmps", bufs=4, space="PSUM") as mmps,
            tc.tile_pool(name="ptps", bufs=2, space="PSUM") as ptps,
            tc.tile_pool(name="yps", bufs=2, space="PSUM") as yps,
        ):
            # ---- load inputs ----
            xT_s = cpool.tile([128, CT, T], BF16)
            wqk_s = cpool.tile([128, CT, 512], BF16)
            wv_s = cpool.tile([128, CT, 256], BF16)
            wp_s = cpool.tile([128, 2, C], BF16)
            bqk_s = cpool.tile([128, 4], F32)
            bv_s = cpool.tile([128, 256], BF16)
            mask_s = cpool.tile([128, 128], F32)
            ident_s = cpool.tile([128, 128], BF16)

            for i in range(CT):
                nc.sync.dma_start(out=wqk_s[:, i, :], in_=wqk_d.ap()[128 * i:128 * (i + 1), :])
                nc.sync.dma_start(out=wv_s[:, i, :], in_=wv_d.ap()[128 * i:128 * (i + 1), :])
            for i in range(2):
                nc.sync.dma_start(out=wp_s[:, i, :], in_=wp_d.ap()[128 * i:128 * (i + 1), :])
            nc.sync.dma_start(out=bqk_s[:], in_=bqk_d.ap()[:])
            nc.sync.dma_start(out=bv_s[:], in_=bv_d.ap()[:])
            nc.sync.dma_start(out=mask_s[:], in_=mask_d.ap()[:])
            nc.sync.dma_start(out=ident_s[:], in_=ident_d.ap()[:])
            for i in range(CT):
                nc.sync.dma_start(out=xT_s[:, i, :], in_=xT_d.ap()[128 * i:128 * (i + 1), :])

            # ---- QKV: qk^T [channel, t] ----
            # jtile 0: Q heads {0,1}; 1: Q heads {2,3}; 2: K heads {0,1}; 3: K heads {2,3}
            qkT_s = qkpool.tile([128, 4, T], BF16)
            for j in range(4):
                for t4 in range(T // 512):
                    ps = mmps.tile([128, 512], F32, tag="mm")
                    for i in range(CT):
                        nc.tensor.matmul(
                            ps[:],
                            wqk_s[:, i, 128 * j:128 * (j + 1)],
                            xT_s[:, i, 512 * t4:512 * (t4 + 1)],
                            start=(i == 0), stop=(i == CT - 1),
                        )
                    nc.vector.tensor_scalar_add(
                        qkT_s[:, j, 512 * t4:512 * (t4 + 1)], ps[:], bqk_s[:, j:j + 1])

            # ---- V: [t, channel] ----
            v_s = vpool.tile([128, TQ, 256], BF16)
            for t in range(TQ):
                ps = mmps.tile([128, 256], F32, tag="mm")
                for i in range(CT):
                    nc.tensor.matmul(
                        ps[:],
                        xT_s[:, i, 128 * t:128 * (t + 1)],
                        wv_s[:, i, :],
                        start=(i == 0), stop=(i == CT - 1),
                    )
                nc.vector.tensor_add(v_s[:, t, :], ps[:], bv_s[:])

            # ---- attention ----
            yT_s = ypool.tile([128, 2, T], BF16)
            for qb in range(TQ):
                span = 128 * (qb + 1)
                offs = list(range(0, span, 512))
                for p in range(2):
                    jq, jk = p, 2 + p
                    qA = qkT_s[0:64, jq, 128 * qb:128 * (qb + 1)]
                    qB = qkT_s[64:128, jq, 128 * qb:128 * (qb + 1)]
                    P_A = ppool.tile([128, T], BF16, tag="PA")
                    P_B = ppool.tile([128, T], BF16, tag="PB")
                    lp_A = lpool.tile([128, 4], F32, tag="lp")
                    lp_B = lpool.tile([128, 4], F32, tag="lp")
                    for ci, off in enumerate(offs):
                        w = min(512, span - off)
                        sA = mmps.tile([128, 512], F32, tag="mm")
                        sB = mmps.tile([128, 512], F32, tag="mm")
                        nc.tensor.matmul(sA[:, :w], qA, qkT_s[0:64, jk, off:off + w],
                                         start=True, stop=True, tile_position=(0, 0))
                        nc.tensor.matmul(sB[:, :w], qB, qkT_s[64:128, jk, off:off + w],
                                         start=True, stop=True, tile_position=(64, 0))
                        if off + w == span:  # chunk containing the diagonal block
                            nc.vector.tensor_add(sA[:, w - 128:w], sA[:, w - 128:w], mask_s[:])
                            nc.vector.tensor_add(sB[:, w - 128:w], sB[:, w - 128:w], mask_s[:])
                        nc.scalar.activation(P_A[:, off:off + w], sA[:, :w], Exp,
                                             scale=SCALE, accum_out=lp_A[:, ci:ci + 1])
                        nc.scalar.activation(P_B[:, off:off + w], sB[:, :w], Exp,
                                             scale=SCALE, accum_out=lp_B[:, ci:ci + 1])
                    ls_A = lpool.tile([128, 1], F32, tag="ls")
                    ls_B = lpool.tile([128, 1], F32, tag="ls")
                    nc.vector.reduce_sum(ls_A, lp_A[:, :len(offs)], axis=X)
                    nc.vector.reduce_sum(ls_B, lp_B[:, :len(offs)], axis=X)
                    r_A = lpool.tile([128, 1], F32, tag="r")
                    r_B = lpool.tile([128, 1], F32, tag="r")
                    nc.vector.reciprocal(r_A, ls_A)
                    nc.vector.reciprocal(r_B, ls_B)
                    nc.vector.tensor_scalar_mul(P_A[:, :span], P_A[:, :span], r_A)
                    nc.vector.tensor_scalar_mul(P_B[:, :span], P_B[:, :span], r_B)

                    # transpose P in 128-blocks, 4 per PSUM tile
                    pts_A, pts_B = [], []
                    for k4 in range(0, qb + 1, 4):
                        nblk = min(4, qb + 1 - k4)
                        for P_x, pts in ((P_A, pts_A), (P_B, pts_B)):
                            pp = ptps.tile([128, 512], BF16, tag="ptp")
                            for m in range(nblk):
                                nc.tensor.transpose(
                                    pp[:, 128 * m:128 * (m + 1)],
                                    P_x[:, 128 * (k4 + m):128 * (k4 + m + 1)],
                                    ident_s[:])
                            sb = ptpool.tile([128, 512], BF16, tag="pt")
                            nc.vector.tensor_copy(sb[:, :128 * nblk], pp[:, :128 * nblk])
                            pts.append(sb)

                    y_ps = yps.tile([128, 128], F32, tag="y")
                    for kb in range(qb + 1):
                        ptA = pts_A[kb // 4][:, 128 * (kb % 4):128 * (kb % 4 + 1)]
                        ptB = pts_B[kb // 4][:, 128 * (kb % 4):128 * (kb % 4 + 1)]
                        nc.tensor.matmul(y_ps[0:64, :], v_s[:, kb, 128 * p:128 * p + 64],
                                         ptA, start=(kb == 0), stop=(kb == qb),
                                         tile_position=(0, 0), skip_group_check=True)
                        nc.tensor.matmul(y_ps[64:128, :], v_s[:, kb, 128 * p + 64:128 * (p + 1)],
                                         ptB, start=(kb == 0), stop=(kb == qb),
                                         tile_position=(0, 64), skip_group_check=True)
                    nc.vector.tensor_copy(yT_s[:, p, 128 * qb:128 * (qb + 1)], y_ps[:])

            # ---- projection (partial: this core's 256 channels) ----
            for t in range(TQ):
                o_t = opool.tile([128, C], F32, tag="o")
                for n in range(2):
                    ps = mmps.tile([128, 512], F32, tag="mm")
                    for p2 in range(2):
                        nc.tensor.matmul(
                            ps[:],
                            yT_s[:, p2, 128 * t:128 * (t + 1)],
                            wp_s[:, p2, 512 * n:512 * (n + 1)],
                            start=(p2 == 0), stop=(p2 == 1),
                        )
                    nc.vector.tensor_copy(o_t[:, 512 * n:512 * (n + 1)], ps[:])
                nc.sync.dma_start(out=out_d.ap()[128 * t:128 * (t + 1), :], in_=o_t[:])

    nc.compile()
    return nc


def _shard_inputs(x, W_attn, b_attn, W_proj, b_proj):
    """Build the 8 per-core input maps (numpy, bf16 where applicable)."""
    mask = np.where(np.tril(np.ones((128, 128), dtype=bool)), 0.0, -1e9).astype(np.float32)
    ident = np.eye(128, dtype=NP_BF16)
    in_maps = []
    for c in range(N_CORES):
        b, g = c // 4, c % 4
        ch = slice(256 * g, 256 * (g + 1))
        wq = W_attn[:, ch]
        wk = W_attn[:, C:][:, ch]
        wv = W_attn[:, 2 * C:][:, ch]
        # jtile ordering: [Q pair0 | Q pair1 | K pair0 | K pair1]
        wqk = np.concatenate([wq, wk], axis=1).astype(NP_BF16)
        bq = b_attn[ch]
        bk = b_attn[C:][ch]
        bv = b_attn[2 * C:][ch]
        bqk = np.concatenate([bq, bk]).reshape(4, 128).T.astype(np.float32)  # [128, 4]
        in_maps.append({
            "xT": np.ascontiguousarray(x[b].T).astype(NP_BF16),
            "wqk": wqk,
            "wv": wv.astype(NP_BF16),
            "wp": W_proj[ch, :].astype(NP_BF16),
            "bqk": np.ascontiguousarray(bqk),
            "bv": np.broadcast_to(bv.astype(NP_BF16), (128, 256)).copy(),
            "maskneg": mask,
            "ident": ident,
        })
    return in_maps


def _run(in_maps, trace=False, **kw):
    global _compiled
    if _compiled is None:
        _compiled = _build_nc()
    return run_bass_kernel_spmd(_compiled, in_maps, list(range(N_CORES)),
                                trace=trace, **kw)


def kernel(x, W_attn, b_attn, W_proj, b_proj):
    x = np.asarray(x, dtype=np.float32)
    W_attn = np.asarray(W_attn, dtype=np.float32)
    b_attn = np.asarray(b_attn, dtype=np.float32)
    W_proj = np.asarray(W_proj, dtype=np.float32)
    b_proj = np.asarray(b_proj, dtype=np.float32)

    in_maps = _shard_inputs(x, W_attn, b_attn, W_proj, b_proj)
    res = _run(in_maps)
    out = np.zeros((B, T, C), dtype=np.float32)
    for c in range(N_CORES):
        out[c // 4] += res.results[c]["out"]
    out += b_proj
    return out


# revision 8
# speedup vs baseline: 1.0103x; 1.0103x over previous
"""Causal self-attention (B=2, T=2048, C=1024, H=16) on 8 trn2 NeuronCores.

Sharding: core c = (batch b = c // 4, head-group g = c % 4). Each core
computes, for its batch, QKV for heads [4g, 4g+4), causal attention, and a
partial output projection through rows [256g, 256g+256) of W_proj. The host
sums the 4 partial projections per batch (tensor-parallel unshard) and adds
b_proj.

Per-core kernel layout choices (all matmul inputs bf16, fp32 PSUM):
  - qk^T is produced transposed ([channel, t]) so attention scores need no
    input transposes (contraction over d=64 sits on the partition axis).
  - Scores are computed TRANSPOSED (S^T[k, q] tiles): exp(S^T) is directly
    the P^T operand the PV matmul needs, so no PE transposes at all (PE
    transposes don't count as HAM activity and kept v1 throttled at 1.2GHz).
  - Two heads are row-packed per S^T matmul (K=64 each at array rows 0/64).
  - V carries an appended ones column (lhsT [128, 65]) so the PV matmul
    accumulates the softmax denominator as row 64 of y^T_aug for free.
  - Normalization: gpsimd partition_broadcast of the denominator row +
    reciprocal + multiply fused into the mandatory PSUM->SBUF copy.
  - y lands transposed ([d, q]), exactly the lhsT the projection needs.
"""

import sys

for _p in ("/opt/trn_rl_repo",):
    if _p not in sys.path:
        sys.path.insert(0, _p)

import numpy as np
import ml_dtypes

import concourse.bass as bass
import concourse.tile as tile
from concourse import bacc, mybir
from concourse.bass_utils import run_bass_kernel_spmd

BF16 = mybir.dt.bfloat16
F32 = mybir.dt.float32
NP_BF16 = ml_dtypes.bfloat16

B, T, C = 2, 2048, 1024
H, D = 16, 64
N_CORES = 8
CT = C // 128   # 8 contraction tiles
TQ = T // 128   # 16 key blocks
QC = T // 512   # 4 query chunks
SCALE = 1.0 / np.sqrt(D)

_compiled = None


def _build_nc(dbg=False):
    nc = bacc.Bacc("TRN2", target_bir_lowering=False, debug=False,
                   enable_asserts=False)
    if dbg:
        dbg_qkT = nc.dram_tensor("dbg_qkT", [128, 4, T], BF16, kind="ExternalOutput")
        dbg_v = nc.dram_tensor("dbg_v", [128, TQ, 4, 65], BF16, kind="ExternalOutput")
        dbg_pt = nc.dram_tensor("dbg_pt", [128, 512], BF16, kind="ExternalOutput")
        dbg_yc = nc.dram_tensor("dbg_yc", [65, 512], F32, kind="ExternalOutput")
        dbg_lb = nc.dram_tensor("dbg_lb", [64, 512], F32, kind="ExternalOutput")
        dbg_yT = nc.dram_tensor("dbg_yT", [128, 2, T], BF16, kind="ExternalOutput")

    xT_d = nc.dram_tensor("xT", [C, T], BF16, kind="ExternalInput")
    wqk_d = nc.dram_tensor("wqk", [C, 512], BF16, kind="ExternalInput")
    wv_d = nc.dram_tensor("wv", [C, 256], BF16, kind="ExternalInput")
    wp_d = nc.dram_tensor("wp", [256, C], BF16, kind="ExternalInput")
    bqk_d = nc.dram_tensor("bqk", [128, 4], F32, kind="ExternalInput")
    bv_d = nc.dram_tensor("bv", [128, 256], BF16, kind="ExternalInput")
    mask_d = nc.dram_tensor("maskT", [128, 128], F32, kind="ExternalInput")
    out_d = nc.dram_tensor("out", [T, C], F32, kind="ExternalOutput")

    Exp = mybir.ActivationFunctionType.Exp

    with tile.TileContext(nc) as tc:
        with (
            tc.tile_pool(name="const", bufs=1) as cpool,
            tc.tile_pool(name="qkT", bufs=1) as qkpool,
            tc.tile_pool(name="vbuf", bufs=1) as vpool,
            tc.tile_pool(name="ybuf", bufs=1) as ypool,
            tc.tile_pool(name="pt", bufs=6) as ptpool,
            tc.tile_pool(name="norm", bufs=6) as npool,
            tc.tile_pool(name="ostage", bufs=3) as opool,
            tc.tile_pool(name="mmps", bufs=5, space="PSUM") as mmps,
            tc.tile_pool(name="yaps", bufs=3, space="PSUM") as yaps,
        ):
            # ---- load inputs ----
            xT_s = cpool.tile([128, CT, T], BF16)
            wqk_s = cpool.tile([128, CT, 512], BF16)
            wv_s = cpool.tile([128, CT, 256], BF16)
            wp_s = cpool.tile([128, 2, C], BF16)
            bqk_s = cpool.tile([128, 4], F32)
            bv_s = cpool.tile([128, 256], BF16)
            mask_s = cpool.tile([128, 128], F32)

            for i in range(CT):
                nc.sync.dma_start(out=wqk_s[:, i, :], in_=wqk_d.ap()[128 * i:128 * (i + 1), :])
                nc.sync.dma_start(out=wv_s[:, i, :], in_=wv_d.ap()[128 * i:128 * (i + 1), :])
            for i in range(2):
                nc.sync.dma_start(out=wp_s[:, i, :], in_=wp_d.ap()[128 * i:128 * (i + 1), :])
            nc.sync.dma_start(out=bqk_s[:], in_=bqk_d.ap()[:])
            nc.sync.dma_start(out=bv_s[:], in_=bv_d.ap()[:])
            nc.sync.dma_start(out=mask_s[:], in_=mask_d.ap()[:])
            for i in range(CT):
                nc.sync.dma_start(out=xT_s[:, i, :], in_=xT_d.ap()[128 * i:128 * (i + 1), :])

            # ---- QKV: qk^T [channel, t] ----
            # jtile 0: Q heads {0,1}; 1: Q heads {2,3}; 2: K heads {0,1}; 3: K heads {2,3}
            qkT_s = qkpool.tile([128, 4, T], BF16)
            for j in range(4):
                for t4 in range(QC):
                    ps = mmps.tile([128, 512], F32, tag="mm")
                    for i in range(CT):
                        nc.tensor.matmul(
                            ps[:],
                            wqk_s[:, i, 128 * j:128 * (j + 1)],
                            xT_s[:, i, 512 * t4:512 * (t4 + 1)],
                            start=(i == 0), stop=(i == CT - 1),
                        )
                    nc.vector.tensor_scalar_add(
                        qkT_s[:, j, 512 * t4:512 * (t4 + 1)], ps[:], bqk_s[:, j:j + 1])

            if dbg:
                nc.sync.dma_start(out=dbg_qkT.ap()[:], in_=qkT_s[:])

            # ---- V (augmented with a ones column per head): [t, 4, 65] ----
            v_s = vpool.tile([128, TQ, 4, 65], BF16)
            nc.vector.memset(v_s[:, :, :, 64:65], 1.0)
            for t in range(TQ):
                ps = mmps.tile([128, 256], F32, tag="mm")
                for i in range(CT):
                    nc.tensor.matmul(
                        ps[:],
                        xT_s[:, i, 128 * t:128 * (t + 1)],
                        wv_s[:, i, :],
                        start=(i == 0), stop=(i == CT - 1),
                    )
                nc.vector.tensor_add(
                    v_s[:, t, :, 0:64],
                    ps[:].rearrange("p (h d) -> p h d", h=4),
                    bv_s[:].rearrange("p (h d) -> p h d", h=4))

            if dbg:
                nc.sync.dma_start(out=dbg_v.ap()[:], in_=v_s[:])

            # ---- attention: S^T tiles [k-block, q-chunk], flash over k ----
            yT_s = ypool.tile([128, 2, T], BF16)
            for qc in range(QC):
                for p in range(2):
                    jq, jk = p, 2 + p
                    nkb = 4 * qc + 4
                    ya = yaps.tile([65, 512], F32, tag="ya")
                    yb = yaps.tile([65, 512], F32, tag="ya")
                    for kb in range(nkb):
                        m = kb - 4 * qc  # >= 0 on the diagonal chunk
                        off = max(m, 0) * 128
                        w = 512 - off
                        for hi, (part, y_ps) in enumerate(((slice(0, 64), ya),
                                                           (slice(64, 128), yb))):
                            tp = (0 if hi == 0 else 64, 0)
                            s_ps = mmps.tile([128, 512], F32, tag="mm")
                            nc.tensor.matmul(
                                s_ps[:, :w],
                                qkT_s[part, jk, 128 * kb:128 * (kb + 1)],
                                qkT_s[part, jq, 512 * qc + off:512 * (qc + 1)],
                                start=True, stop=True, tile_position=tp)
                            if m >= 0:
                                nc.vector.tensor_add(s_ps[:, 0:128], s_ps[:, 0:128], mask_s[:])
                            pt = ptpool.tile([128, 512], BF16, tag="pt")
                            if off:
                                nc.vector.memset(pt[:, :off], 0.0)
                            nc.scalar.activation(pt[:, off:512], s_ps[:, :w], Exp, scale=SCALE)
                            if dbg and qc == 0 and p == 0 and kb == 0 and hi == 0:
                                nc.sync.dma_start(out=dbg_pt.ap()[:], in_=pt[:])
                            nc.tensor.matmul(
                                y_ps[:],
                                v_s[:, kb, 2 * p + hi, :],
                                pt[:],
                                start=(kb == 0), stop=(kb == nkb - 1))
                    # normalize + write y^T (head A -> partitions 0:64, B -> 64:128)
                    for hi, y_ps in ((0, ya), (1, yb)):
                        yc = npool.tile([65, 512], F32, tag="yc")
                        nc.vector.tensor_copy(yc[:], y_ps[:])
                        lb = npool.tile([64, 512], F32, tag="lb")
                        nc.sync.dma_start(
                            out=lb[:],
                            in_=yc[64:65, :].unsqueeze(1).broadcast_to([1, 64, 512]))
                        if dbg and qc == 0 and p == 0 and hi == 0:
                            nc.sync.dma_start(out=dbg_yc.ap()[:], in_=yc[:])
                            nc.sync.dma_start(out=dbg_lb.ap()[:], in_=lb[:])
                        rb = npool.tile([64, 512], F32, tag="rb")
                        nc.vector.reciprocal(rb[:], lb[:])
                        nc.vector.tensor_mul(
                            yT_s[64 * hi:64 * (hi + 1), p, 512 * qc:512 * (qc + 1)],
                            yc[0:64, :], rb[:])

            if dbg:
                nc.sync.dma_start(out=dbg_yT.ap()[:], in_=yT_s[:])

            # ---- projection (partial: this core's 256 channels) ----
            for t in range(TQ):
                o_t = opool.tile([128, C], F32, tag="o")
                for n in range(2):
                    ps = mmps.tile([128, 512], F32, tag="mm")
                    for p2 in range(2):
                        nc.tensor.matmul(
                            ps[:],
                            yT_s[:, p2, 128 * t:128 * (t + 1)],
                            wp_s[:, p2, 512 * n:512 * (n + 1)],
                            start=(p2 == 0), stop=(p2 == 1),
                        )
                    nc.vector.tensor_copy(o_t[:, 512 * n:512 * (n + 1)], ps[:])
                nc.sync.dma_start(out=out_d.ap()[128 * t:128 * (t + 1), :], in_=o_t[:])

    nc.compile()
    return nc


def _shard_inputs(x, W_attn, b_attn, W_proj, b_proj):
    """Build the 8 per-core input maps (numpy, bf16 where applicable)."""
    # S^T diagonal-block mask: entry (p, j) masked where k > q, i.e. p > j
    mask = np.where(np.arange(128)[:, None] > np.arange(128)[None, :],
                    -1e9, 0.0).astype(np.float32)
    in_maps = []
    for c in range(N_CORES):
        b, g = c // 4, c % 4
        ch = slice(256 * g, 256 * (g + 1))
        wq = W_attn[:, ch]
        wk = W_attn[:, C:][:, ch]
        wv = W_attn[:, 2 * C:][:, ch]
        wqk = np.concatenate([wq, wk], axis=1).astype(NP_BF16)
        bq = b_attn[ch]
        bk = b_attn[C:][ch]
        bv = b_attn[2 * C:][ch]
        bqk = np.concatenate([bq, bk]).reshape(4, 128).T.astype(np.float32)  # [128, 4]
        in_maps.append({
            "xT": np.ascontiguousarray(x[b].T).astype(NP_BF16),
            "wqk": wqk,
            "wv": wv.astype(NP_BF16),
            "wp": W_proj[ch, :].astype(NP_BF16),
            "bqk": np.ascontiguousarray(bqk),
            "bv": np.broadcast_to(bv.astype(NP_BF16), (128, 256)).copy(),
            "maskT": mask,
        })
    return in_maps


def _run(in_maps, trace=False, **kw):
    global _compiled
    if _compiled is None:
        _compiled = _build_nc()
    return run_bass_kernel_spmd(_compiled, in_maps, list(range(N_CORES)),
                                trace=trace, **kw)


def kernel(x, W_attn, b_attn, W_proj, b_proj):
    x = np.asarray(x, dtype=np.float32)
    W_attn = np.asarray(W_attn, dtype=np.float32)
    b_attn = np.asarray(b_attn, dtype=np.float32)
    W_proj = np.asarray(W_proj, dtype=np.float32)
    b_proj = np.asarray(b_proj, dtype=np.float32)

    in_maps = _shard_inputs(x, W_attn, b_attn, W_proj, b_proj)
    res = _run(in_maps)
    out = np.zeros((B, T, C), dtype=np.float32)
    for c in range(N_CORES):
        out[c // 4] += res.results[c]["out"]
    out += b_proj
    return out
